# revision 1
# baseline (speedup 1.0000x reference)
"""DoubleGCN Trainium2 kernel: 8 NeuronCores, relation x node-half sharding.

Strategy: 4 relations x 2 node-halves. Edges are bucketed by destination
window (64 nodes) on the host; on device, per-edge messages are gathered
with SWDGE dma_gather and segment-summed via TensorE matmuls against
one-hot S matrices built on VectorE (no scatter needed). Degree, batchnorm
statistics (cross-half AllReduce), and the half-table exchange (AllGather)
all run on device.
"""

# Host-side preprocessing: shard inputs for 8 cores (4 relations x 2 node-halves).
# All numeric model compute stays on device; this only does layout/sharding:
# edge partitioning by dst half, window bucketing, padding, index wrapping.
import numpy as np


class Cfg:
    def __init__(self, N, F, H, O, R, E, B, WIN=64, NG=2048, BPAD=128):
        assert N % 2 == 0
        self.N, self.F, self.H, self.O, self.R, self.E, self.B = N, F, H, O, R, E, B
        self.NHALF = N // 2
        self.WIN = WIN
        self.BANKW = 8           # windows per psum bank group
        bank_nodes = WIN * self.BANKW  # 512
        self.NP = ((self.NHALF + bank_nodes - 1) // bank_nodes) * bank_nodes
        self.NW = self.NP // WIN
        self.NBANK = self.NW // self.BANKW
        self.NG = NG             # idxs per dma_gather call
        self.OP = 64             # padded out dim
        self.BP = ((B + BPAD - 1) // BPAD) * BPAD + BPAD  # padded batch slots (worst-case one half)
        # K (chunks per window per stream) set after scanning data
        self.K = None


def wrap_idx(idx_flat):
    """int16 indices -> [128, L/16] wrapped layout (i -> partition i%16, col i//16),
    replicated across the 8 16-partition groups."""
    L = idx_flat.shape[0]
    assert L % 16 == 0
    w = idx_flat.reshape(L // 16, 16).T  # [16, L/16]
    return np.tile(w, (8, 1)).astype(np.int16)


def prep_edges_core(cfg, srcg, dstg, h):
    """Returns per-stream (gidx int16 [slots*128], drel f32 [slots_per_window layout])"""
    NH, WIN = cfg.NHALF, cfg.WIN
    m = (dstg >= NH) == (h == 1)
    dl = (dstg[m] - h * NH).astype(np.int64)
    sg = srcg[m].astype(np.int64)
    order = np.argsort(dl, kind="stable")
    dl, sg = dl[order], sg[order]
    ma = sg < NH
    streams = [(sg[ma], dl[ma]), (sg[~ma] - NH, dl[~ma])]
    # per-window counts
    out = []
    for sidx, (si, di) in enumerate(streams):
        starts = np.searchsorted(di, np.arange(cfg.NW) * WIN)
        ends = np.searchsorted(di, np.arange(1, cfg.NW + 1) * WIN)
        out.append((si, di, starts, ends))
    return out


def build_core_tensors(cfg, streams, K):
    """Pack streams into fixed K chunks/window layout. Returns gidxA, gidxB (wrapped int16),
    drel [128, NI] f32 (cast to bf16 by caller)."""
    NW, WIN = cfg.NW, cfg.WIN
    NI = NW * 2 * K
    drel = np.full((128, NI), -1.0, np.float32)
    gidxs = []
    for s, (si, di, starts, ends) in enumerate(streams):
        slots = NW * K * 128
        gi = np.zeros(slots, np.int64)
        for w in range(NW):
            lo, hi = starts[w], ends[w]
            cnt = hi - lo
            assert cnt <= K * 128, f"window overflow {cnt} > {K*128}"
            base = w * K * 128
            gi[base:base + cnt] = si[lo:hi]
            # dst_rel for instances
            rel = (di[lo:hi] - w * WIN).astype(np.float32)
            for j in range(K):
                col = w * 2 * K + s * K + j
                seg = rel[j * 128:(j + 1) * 128]
                if len(seg):
                    drel[: len(seg), col] = seg
        # pad stream length to NG multiple
        Lp = ((slots + cfg.NG - 1) // cfg.NG) * cfg.NG
        gi = np.concatenate([gi, np.zeros(Lp - slots, np.int64)])
        gidxs.append(wrap_idx(gi.astype(np.int16)))
    return gidxs[0], gidxs[1], drel


def compute_K(cfg, all_streams):
    mx = 0
    for streams in all_streams:
        for (si, di, starts, ends) in streams:
            c = (ends - starts).max() if len(starts) else 0
            mx = max(mx, int(c))
    return (mx + 127) // 128


def prep_all(cfg, inputs):
    feats = np.asarray(inputs["features"], np.float32)
    edges = np.asarray(inputs["multi_r_edge_index"])
    batch = np.asarray(inputs["batch_nodes"])
    W1 = np.asarray(inputs["W1"], np.float32)
    g1 = np.asarray(inputs["g1"], np.float32)
    be1 = np.asarray(inputs["be1"], np.float32)
    W2 = np.asarray(inputs["W2"], np.float32)
    g2 = np.asarray(inputs["g2"], np.float32)
    be2 = np.asarray(inputs["be2"], np.float32)

    NH, NP, F, H, O, OP = cfg.NHALF, cfg.NP, cfg.F, cfg.H, cfg.O, cfg.OP

    all_streams = []
    for c in range(2 * cfg.R):
        r, h = c // 2, c % 2
        all_streams.append(prep_edges_core(cfg, edges[r][0], edges[r][1], h))
    K = compute_K(cfg, all_streams)
    cfg.K = K

    iota = np.tile(np.arange(cfg.WIN, dtype=np.float32), (128, 1))
    ident = np.eye(128, dtype=np.float32)

    cores = []
    for c in range(2 * cfg.R):
        r, h = c // 2, c % 2
        fh = np.zeros((NP, F), np.float32)
        fh[:NH] = feats[h * NH:(h + 1) * NH]
        gidxA, gidxB, drel = build_core_tensors(cfg, all_streams[c], K)
        # batch split
        bm = (batch >= NH) == (h == 1)
        pos = np.nonzero(bm)[0]
        bl = (batch[bm] - h * NH).astype(np.int64)
        bidx = np.zeros(cfg.BP, np.int64)
        bidx[: len(bl)] = bl
        W2p = np.zeros((H, OP), np.float32)
        W2p[:, :O] = W2[r]
        bn2g = np.zeros((OP, 1), np.float32); bn2g[:O, 0] = g2[r]
        bn2b = np.zeros((OP, 1), np.float32); bn2b[:O, 0] = be2[r]
        cores.append(dict(
            tensors=dict(
                features_h=fh,
                W1=W1[r].copy(),
                W2p=W2p,
                bn1g=g1[r].reshape(H, 1).copy(), bn1b=be1[r].reshape(H, 1).copy(),
                bn2g=bn2g, bn2b=bn2b,
                gidxA=gidxA, gidxB=gidxB,
                drel=drel,          # f32; cast to bf16 at feed time
                bidx=wrap_idx(bidx.astype(np.int16)),
                iota=iota,          # bf16 at feed
                ident=ident,
            ),
            pos=pos, nb=len(bl),
        ))
    return cores


import concourse.bass as bass
import concourse.mybir as mybir
import concourse.tile as tile
import concourse.bacc as bacc

f32 = mybir.dt.float32
bf16 = mybir.dt.bfloat16
i16 = mybir.dt.int16
AF = mybir.ActivationFunctionType
ALU = mybir.AluOpType
EPS = 1e-5


def build(cfg, n_cores=8):
    N, F, H, O, OP = cfg.N, cfg.F, cfg.H, cfg.O, cfg.OP
    NP, NW, WIN, K, NG, BP = cfg.NP, cfg.NW, cfg.WIN, cfg.K, cfg.NG, cfg.BP
    NBLK = NP // 128
    NBANK = NP // 512
    BANKW = cfg.BANKW
    K2 = 2 * K
    NI = NW * K2
    SLOTS = NW * K * 128
    LP = ((SLOTS + NG - 1) // NG) * NG
    CPC = NG // 128
    SG = 8
    PAIRS = [[2 * i, 2 * i + 1] for i in range(max(1, n_cores // 2))]

    nc = bacc.Bacc(None, target_bir_lowering=False)

    feat = nc.dram_tensor("features_h", [NP, F], f32, kind="ExternalInput")
    W1d = nc.dram_tensor("W1", [F, H], f32, kind="ExternalInput")
    W2d = nc.dram_tensor("W2p", [H, OP], f32, kind="ExternalInput")
    bn1g = nc.dram_tensor("bn1g", [H, 1], f32, kind="ExternalInput")
    bn1b = nc.dram_tensor("bn1b", [H, 1], f32, kind="ExternalInput")
    bn2g = nc.dram_tensor("bn2g", [OP, 1], f32, kind="ExternalInput")
    bn2b = nc.dram_tensor("bn2b", [OP, 1], f32, kind="ExternalInput")
    gidxd = [nc.dram_tensor("gidxA", [128, LP // 16], i16, kind="ExternalInput"),
             nc.dram_tensor("gidxB", [128, LP // 16], i16, kind="ExternalInput")]
    dreld = nc.dram_tensor("drel", [128, NI], bf16, kind="ExternalInput")
    bidxd = nc.dram_tensor("bidx", [128, BP // 16], i16, kind="ExternalInput")
    iotad = nc.dram_tensor("iota", [128, WIN], bf16, kind="ExternalInput")
    identd = nc.dram_tensor("ident", [128, 128], f32, kind="ExternalInput")
    outd = nc.dram_tensor("out", [BP, OP], f32, kind="ExternalOutput")

    with tile.TileContext(nc) as tc:
        with (
            tc.tile_pool(name="const", bufs=1) as cp,
            tc.tile_pool(name="dram", bufs=1, space="DRAM") as dp,
            tc.tile_pool(name="gbufp", bufs=1) as gp,
        ):
            # ---- constants ----
            ident = cp.tile([128, 128], f32); nc.sync.dma_start(ident[:], identd[:])
            iota = cp.tile([128, WIN], bf16); nc.sync.dma_start(iota[:], iotad[:])
            drel = cp.tile([128, NI], bf16); nc.sync.dma_start(drel[:], dreld[:])
            W1sb = cp.tile([128, F // 128, H], f32)
            nc.sync.dma_start(W1sb[:], W1d[:].rearrange("(c p) h -> p c h", p=128))
            W2sb = cp.tile([128, OP], f32); nc.sync.dma_start(W2sb[:], W2d[:])
            bn1gt = cp.tile([H, 1], f32); nc.sync.dma_start(bn1gt[:], bn1g[:])
            bn1bt = cp.tile([H, 1], f32); nc.sync.dma_start(bn1bt[:], bn1b[:])
            bn2gt = cp.tile([OP, 1], f32); nc.sync.dma_start(bn2gt[:], bn2g[:])
            bn2bt = cp.tile([OP, 1], f32); nc.sync.dma_start(bn2bt[:], bn2b[:])
            ones_col_bf = cp.tile([128, 1], bf16); nc.vector.memset(ones_col_bf[:], 1.0)
            ones_row = cp.tile([1, 128], f32); nc.vector.memset(ones_row[:], 1.0)
            epst = cp.tile([128, 1], f32); nc.vector.memset(epst[:], EPS)
            dinv_nm = cp.tile([128, NBLK], f32)
            deg_nm = cp.tile([128, NBLK], f32)
            st1 = cp.tile([H, NBANK, 2], f32)
            st2 = cp.tile([OP, NBANK, 2], f32)
            gbuf = gp.tile([128, NP], f32)

            # internal DRAM
            g1half = dp.tile([NP, H], f32)
            g1tab = dp.tile([2, NP, H], f32)
            g2half = dp.tile([NP, OP], f32)
            g2tab = dp.tile([2, NP, OP], f32)
            x2pre = dp.tile([NP, OP], f32)
            st1l = dp.tile([H, 2], f32)
            st1gl = dp.tile([H, 2], f32)
            st2l = dp.tile([OP, 2], f32)
            st2gl = dp.tile([OP, 2], f32)

            def build_S(pool, g0, dt):
                S = pool.tile([128, SG, WIN], dt, tag="S")
                nc.vector.tensor_tensor(
                    S[:],
                    drel[:, g0:g0 + SG].unsqueeze(2).broadcast_to([128, SG, WIN]),
                    iota[:].unsqueeze(1).broadcast_to([128, SG, WIN]),
                    ALU.is_equal)
                return S

            # helper: dinv broadcast pattern [parts, 128*nq] psum tile
            def dinv_pattern(pat_pool, rowp_pool, rowsb_pool, c0, nq, parts):
                pat = pat_pool.tile([128, 512], f32, tag="dpat")
                for q in range(nq):
                    rowp = rowp_pool.tile([128, 128], f32, tag="rowp")
                    nc.tensor.matmul(rowp[0:1, :], dinv_nm[:, c0 + q:c0 + q + 1],
                                     ident[:], start=True, stop=True)
                    rowsb = rowsb_pool.tile([1, 128], f32, tag="rowsb")
                    nc.scalar.activation(rowsb[:], rowp[0:1, :], AF.Copy)
                    nc.tensor.matmul(pat[0:parts, 128 * q:128 * q + 128],
                                     ones_row[:, 0:parts], rowsb[:], start=True, stop=True)
                return pat

            # ============ DEG PASS ============
            with (
                tc.tile_pool(name="sdeg", bufs=3) as sp,
                tc.tile_pool(name="psdeg", bufs=2, space="PSUM") as psd,
            ):
                Ssup, g0 = None, 0
                for b in range(NBANK):
                    degp = psd.tile([128, 4], f32, tag="degp")
                    for j in range(BANKW):
                        w = b * BANKW + j
                        for i in range(K2):
                            gi = w * K2 + i
                            if gi % SG == 0:
                                Ssup, g0 = build_S(sp, gi, bf16), gi
                            base = 64 * (j % 2)
                            nc.tensor.matmul(
                                degp[base:base + 64, j // 2:j // 2 + 1],
                                Ssup[:, gi - g0, :], ones_col_bf[:],
                                start=(i == 0), stop=(i == K2 - 1))
                    nc.vector.tensor_copy(deg_nm[:, 4 * b:4 * b + 4], degp[:])
                nc.scalar.activation(deg_nm[:], deg_nm[:], AF.Sqrt, bias=1.0)
                nc.vector.reciprocal(dinv_nm[:], deg_nm[:])

            # ============ H1 + G PHASE ============
            with (
                tc.tile_pool(name="h1sb", bufs=3) as hp,
                tc.tile_pool(name="h1tr", bufs=2, space="PSUM") as pstr,
                tc.tile_pool(name="h1mm", bufs=2, space="PSUM") as psmm,
                tc.tile_pool(name="h1bc", bufs=2, space="PSUM") as psbc,
            ):
                for t in range(NBLK):
                    ft = hp.tile([128, F], f32, tag="ft")
                    nc.sync.dma_start(ft[:], feat[128 * t:128 * t + 128, :])
                    xT = hp.tile([128, F // 128, 128], f32, tag="xT")
                    for k in range(F // 128):
                        trp = pstr.tile([128, 128], f32, tag="trp")
                        nc.tensor.transpose(trp[:], ft[:, 128 * k:128 * k + 128], ident[:])
                        nc.scalar.activation(xT[:, k, :], trp[:], AF.Copy)
                    h1p = psmm.tile([128, 128], f32, tag="h1p")
                    for k in range(F // 128):
                        nc.tensor.matmul(h1p[:], W1sb[:, k, :], xT[:, k, :],
                                         start=(k == 0), stop=(k == F // 128 - 1))
                    rowp = pstr.tile([128, 128], f32, tag="trp")
                    nc.tensor.matmul(rowp[0:1, :], dinv_nm[:, t:t + 1], ident[:],
                                     start=True, stop=True)
                    rowsb = hp.tile([1, 128], f32, tag="rowsb")
                    nc.scalar.activation(rowsb[:], rowp[0:1, :], AF.Copy)
                    bcp = psbc.tile([128, 128], f32, tag="bcp")
                    nc.tensor.matmul(bcp[:], ones_row[:], rowsb[:], start=True, stop=True)
                    dsb = hp.tile([128, 128], f32, tag="dsb")
                    nc.scalar.activation(dsb[:], bcp[:], AF.Copy)
                    nc.vector.tensor_tensor(gbuf[:, 128 * t:128 * t + 128], h1p[:],
                                            dsb[:], ALU.mult)
                    trg = pstr.tile([128, 128], f32, tag="trp")
                    nc.tensor.transpose(trg[:], gbuf[:, 128 * t:128 * t + 128], ident[:])
                    gst = hp.tile([128, 128], f32, tag="gst")
                    nc.vector.tensor_copy(gst[:], trg[:])
                    nc.sync.dma_start(g1half[128 * t:128 * t + 128, :], gst[:])

            nc.gpsimd.collective_compute(
                "AllGather", ALU.bypass, replica_groups=PAIRS,
                ins=[g1half.opt()], outs=[g1tab.opt()])

            # ============ SEG PASS (shared L1/L2) ============
            def seg_pass(tab_views, elem, m_parts, finalize):
                with (
                    tc.tile_pool(name="segsb", bufs=3) as sp,
                    tc.tile_pool(name="gat", bufs=3) as gpp,
                    tc.tile_pool(name="idxp", bufs=3) as ip,
                    tc.tile_pool(name="psseg", bufs=2, space="PSUM") as pss,
                    tc.tile_pool(name="pspat", bufs=2, space="PSUM") as psp2,
                    tc.tile_pool(name="pssm", bufs=2, space="PSUM") as pssm,
                    tc.tile_pool(name="psfin", bufs=1, space="PSUM") as psf,
                ):
                    issued = [dict(), dict()]

                    def payload(s, chunk):
                        ci = chunk // CPC
                        if ci not in issued[s]:
                            idxt = ip.tile([128, NG // 16], i16, tag="idx")
                            nc.sync.dma_start(
                                idxt[:], gidxd[s][:, ci * (NG // 16):(ci + 1) * (NG // 16)])
                            gt = gpp.tile([128, CPC, elem], f32, tag="gt")
                            nc.gpsimd.dma_gather(gt[:], tab_views[s], idxt[:], NG, NG, elem, single_packet=False)
                            issued[s] = {ci: gt}
                        return issued[s][ci][:, chunk % CPC, :]

                    Ssup, g0 = None, 0
                    for b in range(NBANK):
                        segp = pss.tile([128, 512], f32, tag="seg")
                        for j in range(BANKW):
                            w = b * BANKW + j
                            for i in range(K2):
                                gi = w * K2 + i
                                if gi % SG == 0:
                                    Ssup, g0 = build_S(sp, gi, f32), gi
                                s, jj = (0, i) if i < K else (1, i - K)
                                pl = payload(s, w * K + jj)
                                nc.tensor.matmul(
                                    segp[0:m_parts, 64 * j:64 * j + 64],
                                    pl, Ssup[:, gi - g0, :],
                                    start=(i == 0), stop=(i == K2 - 1))
                        finalize(b, segp, dict(sb=sp, pat=psp2, sm=pssm, fin=psf))

            # ============ L1 PAYLOAD ============
            def fin1(b, segp, pools):
                pat = dinv_pattern(pools["pat"], pools["sm"], pools["sb"], 4 * b, 4, 128)
                sl = gbuf[:, 512 * b:512 * b + 512]
                nc.vector.tensor_tensor(sl, segp[:], sl, ALU.add)
                nc.vector.tensor_tensor(sl, sl, pat[:], ALU.mult)
                scr = pools["sb"].tile([128, 512], f32, tag="scr")
                nc.vector.tensor_reduce(st1[:, b, 0:1], sl, mybir.AxisListType.X, ALU.add)
                nc.scalar.activation(scr[:], sl, AF.Square)
                nc.vector.tensor_reduce(st1[:, b, 1:2], scr[:], mybir.AxisListType.X, ALU.add)

            seg_pass([g1tab[0], g1tab[1]], H, 128, fin1)

            # ============ BN1 ============
            with tc.tile_pool(name="bnsb", bufs=1) as bp:
                st = bp.tile([H, 2], f32)
                nc.vector.tensor_reduce(st[:, 0:1], st1[:, :, 0], mybir.AxisListType.X, ALU.add)
                nc.vector.tensor_reduce(st[:, 1:2], st1[:, :, 1], mybir.AxisListType.X, ALU.add)
                nc.sync.dma_start(st1l[:], st[:])
                nc.gpsimd.collective_compute(
                    "AllReduce", ALU.add, replica_groups=PAIRS,
                    ins=[st1l.opt()], outs=[st1gl.opt()])
                stg = bp.tile([H, 2], f32)
                nc.sync.dma_start(stg[:], st1gl[:])
                mean = bp.tile([H, 1], f32)
                nc.vector.tensor_scalar_mul(mean[:], stg[:, 0:1], 1.0 / N)
                var = bp.tile([H, 1], f32)
                nc.vector.tensor_scalar_mul(var[:], stg[:, 1:2], 1.0 / N)
                msq = bp.tile([H, 1], f32)
                nc.vector.tensor_tensor(msq[:], mean[:], mean[:], ALU.mult)
                nc.vector.tensor_tensor(var[:], var[:], msq[:], ALU.subtract)
                nc.scalar.activation(var[:], var[:], AF.Sqrt, bias=epst[0:H, :])
                rstd = bp.tile([H, 1], f32)
                nc.vector.reciprocal(rstd[:], var[:])
                sc = bp.tile([H, 1], f32)
                nc.vector.tensor_tensor(sc[:], bn1gt[:], rstd[:], ALU.mult)
                sh = bp.tile([H, 1], f32)
                nc.vector.tensor_tensor(sh[:], mean[:], sc[:], ALU.mult)
                nc.vector.tensor_tensor(sh[:], bn1bt[:], sh[:], ALU.subtract)
                nc.scalar.activation(gbuf[:], gbuf[:], AF.Relu, bias=sh[:], scale=sc[:])
                if NP > cfg.NHALF:
                    nc.vector.memset(gbuf[:, cfg.NHALF:], 0.0)

            # ============ L2 TABLE ============
            with (
                tc.tile_pool(name="l2sb", bufs=3) as lp,
                tc.tile_pool(name="l2h2", bufs=2, space="PSUM") as psl,
                tc.tile_pool(name="l2pat", bufs=2, space="PSUM") as psp3,
                tc.tile_pool(name="l2sm", bufs=2, space="PSUM") as pls,
            ):
                for b in range(NBANK):
                    h2p = psl.tile([64, 512], f32, tag="h2p")
                    nc.tensor.matmul(h2p[:], W2sb[:], gbuf[:, 512 * b:512 * b + 512],
                                     start=True, stop=True)
                    pat = dinv_pattern(psp3, pls, lp, 4 * b, 4, 64)
                    psb = lp.tile([64, 512], f32, tag="psb")
                    nc.scalar.activation(psb[:], pat[0:64, :], AF.Copy)
                    g2sb = lp.tile([64, 512], f32, tag="g2sb")
                    nc.vector.tensor_tensor(g2sb[:], h2p[:], psb[:], ALU.mult)
                    g2nm = lp.tile([128, 4, OP], f32, tag="g2nm")
                    for q in range(4):
                        trp = pls.tile([128, 128], f32, tag="rowp")
                        nc.tensor.transpose(trp[0:128, 0:64],
                                            g2sb[:, 128 * q:128 * q + 128],
                                            ident[0:64, 0:64])
                        nc.vector.tensor_copy(g2nm[:, q, :], trp[0:128, 0:64])
                    nc.sync.dma_start(
                        g2half[512 * b:512 * b + 512, :].rearrange("(q p) e -> p q e", p=128),
                        g2nm[:])

            nc.gpsimd.collective_compute(
                "AllGather", ALU.bypass, replica_groups=PAIRS,
                ins=[g2half.opt()], outs=[g2tab.opt()])

            # ============ L2 PAYLOAD ============
            def fin2(b, segp, pools):
                pat = dinv_pattern(pools["pat"], pools["sm"], pools["sb"], 4 * b, 4, 64)
                psb = pools["sb"].tile([64, 512], f32, tag="psb")
                nc.scalar.activation(psb[:], pat[0:64, :], AF.Copy)
                h2p = pools["fin"].tile([64, 512], f32, tag="h2p2")
                nc.tensor.matmul(h2p[:], W2sb[:], gbuf[:, 512 * b:512 * b + 512],
                                 start=True, stop=True)
                g2sb = pools["sb"].tile([64, 512], f32, tag="g2sb")
                nc.vector.tensor_tensor(g2sb[:], h2p[:], psb[:], ALU.mult)
                x2 = pools["sb"].tile([64, 512], f32, tag="x2")
                nc.vector.tensor_tensor(x2[:], segp[0:64, :], g2sb[:], ALU.add)
                nc.vector.tensor_tensor(x2[:], x2[:], psb[:], ALU.mult)
                nc.vector.tensor_reduce(st2[:, b, 0:1], x2[:], mybir.AxisListType.X, ALU.add)
                scr = pools["sb"].tile([64, 512], f32, tag="scr")
                nc.scalar.activation(scr[:], x2[:], AF.Square)
                nc.vector.tensor_reduce(st2[:, b, 1:2], scr[:], mybir.AxisListType.X, ALU.add)
                x2nm = pools["sb"].tile([128, 4, OP], f32, tag="x2nm")
                for q in range(4):
                    trp = pools["sm"].tile([128, 128], f32, tag="rowp")
                    nc.tensor.transpose(trp[0:128, 0:64], x2[:, 128 * q:128 * q + 128],
                                        ident[0:64, 0:64])
                    nc.vector.tensor_copy(x2nm[:, q, :], trp[0:128, 0:64])
                nc.sync.dma_start(
                    x2pre[512 * b:512 * b + 512, :].rearrange("(q p) e -> p q e", p=128),
                    x2nm[:])

            seg_pass([g2tab[0], g2tab[1]], OP, 64, fin2)

            # ============ BN2 + FINAL ============
            with (
                tc.tile_pool(name="fsb", bufs=2) as fp2,
                tc.tile_pool(name="fps", bufs=1, space="PSUM") as pf,
            ):
                st = fp2.tile([OP, 2], f32, tag="stf")
                nc.vector.tensor_reduce(st[:, 0:1], st2[:, :, 0], mybir.AxisListType.X, ALU.add)
                nc.vector.tensor_reduce(st[:, 1:2], st2[:, :, 1], mybir.AxisListType.X, ALU.add)
                nc.sync.dma_start(st2l[:], st[:])
                nc.gpsimd.collective_compute(
                    "AllReduce", ALU.add, replica_groups=PAIRS,
                    ins=[st2l.opt()], outs=[st2gl.opt()])
                stg = fp2.tile([OP, 2], f32, tag="stg")
                nc.sync.dma_start(stg[:], st2gl[:])
                mean = fp2.tile([OP, 1], f32, tag="mean")
                nc.vector.tensor_scalar_mul(mean[:], stg[:, 0:1], 1.0 / N)
                var = fp2.tile([OP, 1], f32, tag="var")
                nc.vector.tensor_scalar_mul(var[:], stg[:, 1:2], 1.0 / N)
                msq = fp2.tile([OP, 1], f32, tag="msq")
                nc.vector.tensor_tensor(msq[:], mean[:], mean[:], ALU.mult)
                nc.vector.tensor_tensor(var[:], var[:], msq[:], ALU.subtract)
                nc.scalar.activation(var[:], var[:], AF.Sqrt, bias=epst[0:OP, :])
                rstd = fp2.tile([OP, 1], f32, tag="rstd")
                nc.vector.reciprocal(rstd[:], var[:])
                sc = fp2.tile([OP, 1], f32, tag="sc")
                nc.vector.tensor_tensor(sc[:], bn2gt[:], rstd[:], ALU.mult)
                sh = fp2.tile([OP, 1], f32, tag="sh")
                nc.vector.tensor_tensor(sh[:], mean[:], sc[:], ALU.mult)
                nc.vector.tensor_tensor(sh[:], bn2bt[:], sh[:], ALU.subtract)
                reps = []
                for vi, v in enumerate((sc, sh)):
                    rowp = pf.tile([128, 128], f32, tag="rowp")
                    nc.tensor.matmul(rowp[0:1, 0:OP], v[:], ident[0:OP, 0:OP],
                                     start=True, stop=True)
                    rowsb = fp2.tile([1, OP], f32, tag=f"rowsb{vi}")
                    nc.scalar.activation(rowsb[:], rowp[0:1, 0:OP], AF.Copy)
                    repp = pf.tile([128, OP], f32, tag=f"rep{vi}")
                    nc.tensor.matmul(repp[:], ones_row[:], rowsb[:], start=True, stop=True)
                    rep = fp2.tile([128, OP], f32, tag=f"repsb{vi}")
                    nc.vector.tensor_copy(rep[:], repp[:])
                    reps.append(rep)
                sc_rep, sh_rep = reps

                bidxt = fp2.tile([128, BP // 16], i16, tag="bidx")
                nc.sync.dma_start(bidxt[:], bidxd[:])
                NBB = BP // 128
                xb = fp2.tile([128, NBB, OP], f32, tag="xb")
                nc.gpsimd.dma_gather(xb[:], x2pre[:], bidxt[:], BP, BP, OP, single_packet=False)
                nc.vector.tensor_tensor(
                    xb[:], xb[:], sc_rep[:].unsqueeze(1).broadcast_to([128, NBB, OP]), ALU.mult)
                nc.vector.tensor_tensor(
                    xb[:], xb[:], sh_rep[:].unsqueeze(1).broadcast_to([128, NBB, OP]), ALU.add)
                nc.scalar.activation(xb[:], xb[:], AF.Relu)
                xs = xb[:, :, 0:O]
                mx = fp2.tile([128, NBB, 1], f32, tag="mx")
                nc.vector.tensor_reduce(mx[:], xs, mybir.AxisListType.X, ALU.max)
                nc.vector.tensor_tensor(xs, xs, mx[:].broadcast_to([128, NBB, O]), ALU.subtract)
                ex = fp2.tile([128, NBB, O], f32, tag="ex")
                nc.scalar.activation(ex[:], xs, AF.Exp)
                sm = fp2.tile([128, NBB, 1], f32, tag="sm")
                nc.vector.tensor_reduce(sm[:], ex[:], mybir.AxisListType.X, ALU.add)
                nc.scalar.activation(sm[:], sm[:], AF.Ln)
                nc.vector.tensor_tensor(xs, xs, sm[:].broadcast_to([128, NBB, O]), ALU.subtract)
                nc.sync.dma_start(outd[:].rearrange("(g p) e -> p g e", p=128), xb[:])

    nc.compile()
    return nc


import ml_dtypes

TRACE = False
LAST = {"exec_time_ns": None}
_CACHE = {}


def _get_program(cfg):
    key = (cfg.N, cfg.F, cfg.H, cfg.O, cfg.R, cfg.E, cfg.B, cfg.K)
    if key not in _CACHE:
        _CACHE[key] = build(cfg, n_cores=8)
    return _CACHE[key]


def kernel(**inputs):
    from concourse.bass_utils import run_bass_kernel_spmd

    inputs = {k: np.asarray(v) for k, v in inputs.items()}
    N, F = inputs["features"].shape
    R, _, E = inputs["multi_r_edge_index"].shape
    B = inputs["batch_nodes"].shape[0]
    H = inputs["W1"].shape[2]
    O = inputs["W2"].shape[2]
    cfg = Cfg(N=N, F=F, H=H, O=O, R=R, E=E, B=B)
    cores = prep_all(cfg, inputs)
    nc = _get_program(cfg)

    in_maps = []
    for c in range(2 * R):
        t = cores[c]["tensors"]
        in_maps.append(dict(
            features_h=t["features_h"], W1=t["W1"], W2p=t["W2p"],
            bn1g=t["bn1g"], bn1b=t["bn1b"], bn2g=t["bn2g"], bn2b=t["bn2b"],
            gidxA=t["gidxA"], gidxB=t["gidxB"],
            drel=t["drel"].astype(ml_dtypes.bfloat16),
            bidx=t["bidx"], iota=t["iota"].astype(ml_dtypes.bfloat16),
            ident=t["ident"],
        ))

    res = run_bass_kernel_spmd(nc, in_maps, core_ids=list(range(2 * R)), trace=TRACE)
    LAST["exec_time_ns"] = res.exec_time_ns
    LAST["results"] = res

    out = np.zeros((B, R * O), np.float32)
    for c in range(2 * R):
        core = cores[c]
        r = c // 2
        row = np.asarray(res.results[c]["out"])
        out[core["pos"], r * O:(r + 1) * O] = row[:core["nb"], :O]
    return out

